# revision 29
# baseline (speedup 1.0000x reference)
"""Trainium2 Bass kernel for nn_NeuralODEBlock (RK4 neural ODE, 1024->64->1024 MLP).

Strategy
--------
Data parallel over batch: core b gets x[b] (2048 tokens), params replicated.

Low-rank reformulation in 64-dim h-space with a TELESCOPED running sum:
  u0   = W1m^T x
  u_n  = u0 + (h/6) G^T S_{n-1}        (time/c0 terms folded into btab)
  S_n  = S_{n-1} + h1 + 2 h2 + 2 h3 + h4   (PSUM accumulator, one group
                                            spanning all 6 steps)
  z_N  = x + (h/6) W2^T S_5 + b2

Per step n (per paired chunk):
  tanh1 reads the P-bank (holds u_n built at the end of step n-1)
  stages 2..4: fresh bank = I^T u_sb (off-chain) + c_s G^T h_{s-1}; tanh
  S-bank += w_s h_s (identity-weight matmuls, 6-step accumulation group)
  step boundary: S123_sb = copy(S-bank mid-group, = S_{n-1}+h1+2h2+2h3);
  P_{n+1} = I^T u0_sb + (h/6) G^T S123_sb + (h/6) G^T h4
  (only the G^T h4 matmul sits on the tanh chain)

Finale: per (d-slice m, chunk c): pz = W2aug^T sb_c, then the skip add
z = pz + x as a DVE tensor-add that merges the mandatory PSUM->SBUF copy
with the skip connection (measured faster than splitting across a PE
identity-matmul + ACT-copy path: PE is the binding engine). b2 enters via the sb ones-row. z leaves in eight 1MB
DMAs ([128, 2048] zs tiles); x arrives in two 2MB DMAs — small DMAs run
well below peak HBM bandwidth, and only the SP/ACT HWDGE rings are used
(gpsimd dma_start is software-DGE on the Q7 cores).

Everything is bf16 (x, h, weights) except PSUM (f32), biases (f32) and the
z output (f32, dtype-preserving). u0 accumulates directly into the paired
P-bank using output base_partition 64 for odd chunks (legal for bf16).
Layout: feature-major x^T reordered to [128, 8*2048] per core; h-space
tensors are "paired" [128, 512] tiles (rows 0:64 chunk 2p, 64:128 chunk
2p+1) driven by block-diagonal [128,128] weights.

Timing note: For_i iterations are fenced by an all-engine barrier (the
loop's semaphore-reset block), so the timing variant packs UNROLL=4 bodies
per iteration and uses staggered_reset=True; test.py divides the marginal
time by iterations*UNROLL.
"""

import numpy as np

D = 1024
HID = 64
N_STEPS = 6
N_CORES = 8
TOK = 2048          # tokens per core
CH = 512            # token chunk (matmul N)
NCHUNK = TOK // CH  # 4
NPAIR = NCHUNK // 2 # 2
KD = D // 128       # 8 d-chunks

_H = 1.0 / 6.0      # RK4 step size (T1-T0)/N_STEPS


def _blk(m):
    """Block-diagonal duplicate [64,64] -> [128,128]."""
    z = np.zeros((128, 128))
    z[0:HID, 0:HID] = m
    z[HID:128, HID:128] = m
    return z


def _dup(m):
    """Row-stack duplicate [64, x] -> [128, x]."""
    return np.concatenate([m, m], axis=0)


def _bf16(a):
    import ml_dtypes
    return np.ascontiguousarray(np.asarray(a, np.float32).astype(ml_dtypes.bfloat16))


def host_prep(W1, b1, W2, b2):
    """Precompute weight-derived constants (fp64 -> fp32/bf16)."""
    W1 = np.asarray(W1, np.float64)
    b1 = np.asarray(b1, np.float64)
    W2 = np.asarray(W2, np.float64)
    b2 = np.asarray(b2, np.float64)
    W1m = W1[:-1]            # [1024, 64]
    W1t = W1[-1]             # [64]
    G = W2 @ W1m             # [64, 64]
    c0 = W1m.T @ b2          # [64]
    h = _H

    # bias table: column e = 4*n + s; includes the n*h*c0 drift so the
    # PSUM-carried u stays bias-free
    btab = np.zeros((HID, 4 * N_STEPS))
    coffs = [0.0, h / 2, h / 2, h]
    for n in range(N_STEPS):
        tn = n * h
        for s in range(4):
            btab[:, 4 * n + s] = (tn + coffs[s]) * W1t + b1 + coffs[s] * c0 + n * h * c0

    I64 = np.eye(HID)
    bf = {
        # w1 rearranged so SBUF tile [128, 8*64] has k-chunk k at cols 64k:64k+64
        "w1": np.ascontiguousarray(
            W1m.reshape(KD, 128, HID).transpose(1, 0, 2).reshape(128, KD * HID)
        ),
        "gw_half": _blk((h / 2) * G),      # stage 2,3 coefficient
        "gw_full": _blk(h * G),            # stage 4 coefficient
        "gw_sixth": _blk((h / 6) * G),     # pre1 coefficient
        "iw_one": _blk(I64),               # S-acc w=1, stage-init, pre1-init
        "iw_two": _blk(2.0 * I64),         # S-acc w=2
        "w2aug": np.concatenate([(h / 6) * W2, b2[None, :]], axis=0),  # [65, 1024]
        "onesrow": np.ones((1, CH)),
    }
    out = {k: _bf16(v) for k, v in bf.items()}
    out["btab"] = np.ascontiguousarray(_dup(btab), np.float32)  # [128, 24]
    return out


def build_program(loop_iters=None, unroll=1, staggered=False):
    """Build the per-core Bacc program. loop_iters wraps the body in a
    hardware For_i for timing amplification (None = straight-line); with
    unroll=U the loop body holds U full kernel executions so the For_i
    all-engine barrier amortizes (total executions = loop_iters * U).
    unroll with loop_iters=None emits N straight-line bodies (TimelineSim
    devloop). staggered=True uses For_i(staggered_reset=True)."""
    import contextlib
    import concourse.mybir as mybir
    import concourse.tile as tile
    from concourse import bacc

    f32 = mybir.dt.float32
    bf16 = mybir.dt.bfloat16
    TANH = mybir.ActivationFunctionType.Tanh

    nc = bacc.Bacc("TRN2", target_bir_lowering=False, debug=False,
                   num_devices=N_CORES)

    # x reordered host-side to [128, KD*TOK]: k-chunk k lives at columns
    # k*TOK:(k+1)*TOK, so the whole input moves in ONE 4MB DMA (small DMAs
    # run at <60% of HBM bandwidth).
    xt = nc.dram_tensor("xt", [128, KD * TOK], bf16, kind="ExternalInput").ap()
    dr = {}
    for name, shape in [
        ("w1", [128, KD * HID]), ("gw_half", [128, 128]), ("gw_full", [128, 128]),
        ("gw_sixth", [128, 128]), ("iw_one", [128, 128]), ("iw_two", [128, 128]),
        ("w2aug", [HID + 1, D]), ("onesrow", [1, CH]),
    ]:
        dr[name] = nc.dram_tensor(name, shape, bf16, kind="ExternalInput").ap()
    dr["btab"] = nc.dram_tensor("btab", [128, 4 * N_STEPS], f32,
                                kind="ExternalInput").ap()
    zt = nc.dram_tensor("zt", [D, TOK], f32, kind="ExternalOutput").ap()

    with tile.TileContext(nc) as tc, contextlib.ExitStack() as ctx:
        consts = ctx.enter_context(tc.tile_pool(name="consts", bufs=1))
        xpool = ctx.enter_context(tc.tile_pool(name="x", bufs=3))
        spool = ctx.enter_context(tc.tile_pool(name="s", bufs=2))   # u0/u/S123 sbuf
        hpool = ctx.enter_context(tc.tile_pool(name="h", bufs=8))
        sbpool = ctx.enter_context(tc.tile_pool(name="sb", bufs=1))
        zspool = ctx.enter_context(tc.tile_pool(name="zs", bufs=4))
        # PSUM (8 banks): P per pair (2), S per pair (2), stage rotating (2),
        # finale pz (2). Separate pz pool so the finale doesn't serialize the
        # next iteration's RK4 stage banks.
        ps_p = ctx.enter_context(tc.tile_pool(name="ps_p", bufs=1, space="PSUM"))
        ps_s = ctx.enter_context(tc.tile_pool(name="ps_s", bufs=1, space="PSUM"))
        ps_stage = ctx.enter_context(tc.tile_pool(name="ps_stage", bufs=2, space="PSUM"))
        ps_z = ctx.enter_context(tc.tile_pool(name="ps_z", bufs=2, space="PSUM"))

        # ---- constants into SBUF ----
        cs = {}
        lazy = ["gw_half", "gw_full", "gw_sixth", "iw_one", "iw_two",
                "w2aug", "onesrow"]
        for name in ["w1", "btab"] + lazy:
            ap = dr[name]
            t = consts.tile(list(ap.shape), ap.dtype, tag=name, name=f"c_{name}")
            if name in ("w1", "btab"):
                nc.sync.dma_start(out=t[:], in_=ap[:])
            cs[name] = t
        gw_stage = {1: cs["gw_half"], 2: cs["gw_half"], 3: cs["gw_full"]}
        iw_stage = {0: cs["iw_one"], 1: cs["iw_two"],
                    2: cs["iw_two"], 3: cs["iw_one"]}

        # persistent sb tiles: the ones-row (b2 channel) is written once and
        # survives across iterations (copies only touch rows 0:64)
        sbs = []
        for c in range(NCHUNK):
            sb = sbpool.tile([HID + 1, CH], bf16, tag=f"sb{c}", name=f"sb{c}")
            sbs.append(sb)

        dma_engines = [nc.sync, nc.scalar]

        def body(_iv=None):
            # ---- x in: two 2MB DMAs, one per HWDGE ring ----
            xtile = xpool.tile([128, KD * TOK], bf16, tag="x", name="x")
            half_cols = KD * TOK // 2
            nc.scalar.dma_start(out=xtile[:, 0:half_cols], in_=xt[:, 0:half_cols])
            nc.sync.dma_start(out=xtile[:, half_cols:], in_=xt[:, half_cols:])
            if not body.consts_loaded:
                for i, name in enumerate(lazy):
                    eng = dma_engines[i % len(dma_engines)]
                    eng.dma_start(out=cs[name][:], in_=dr[name][:])
                for c in range(NCHUNK):
                    nc.sync.dma_start(out=sbs[c][HID:HID + 1, :],
                                      in_=dr["onesrow"][:])

            def xsl(k, c):
                return xtile[:, k * TOK + c * CH:k * TOK + (c + 1) * CH]

            # ---- u0 = W1^T x straight into the P-banks (paired) ----
            pbank = [ps_p.tile([128, CH], f32, tag=f"P{p}", name=f"P{p}")
                     for p in range(NPAIR)]
            for p in range(NPAIR):
                for half in range(2):
                    c = 2 * p + half
                    out = pbank[p][half * HID:(half + 1) * HID, :]
                    for k in range(KD):
                        w = cs["w1"][:, k * HID:(k + 1) * HID]
                        nc.tensor.matmul(out, w, xsl(k, c),
                                         start=(k == 0), stop=(k == KD - 1),
                                         skip_group_check=True)

            u0sb = [spool.tile([128, CH], bf16, tag=f"u0_{p}", name=f"u0_{p}")
                    for p in range(NPAIR)]
            sbank = [None] * NPAIR

            # ---- RK4 in h-space (paired [128, CH] tiles) ----
            for n in range(N_STEPS):
                last = n == N_STEPS - 1
                hprev = [None] * NPAIR
                h4t = [None] * NPAIR
                s123 = [None] * NPAIR
                pnext = [None] * NPAIR
                usb = [None] * NPAIR
                for s in range(4):
                    e = 4 * n + s
                    bias = cs["btab"][:, e:e + 1]
                    for p in range(NPAIR):
                        ht = hpool.tile([128, CH], bf16, tag="h", name="ht")
                        if s == 0:
                            # u snapshot for this step's stage-inits (off the
                            # tanh chain: P-bank is complete before tanh1).
                            # Step 0's copy runs on ACT so it doesn't queue
                            # behind the previous iteration's finale adds.
                            if n == 0:
                                usb[p] = u0sb[p]
                                nc.scalar.copy(usb[p][:], pbank[p][:])
                            else:
                                usb[p] = spool.tile([128, CH], bf16,
                                                    tag=f"u_{p}", name=f"u_{p}")
                                # ACT, not DVE: DVE serves the overlapped
                                # previous body's finale adds during early RK4
                                nc.scalar.copy(usb[p][:], pbank[p][:])
                            nc.scalar.activation(ht[:], pbank[p][:], TANH,
                                                 bias=bias)
                        else:
                            ps = ps_stage.tile([128, CH], f32, tag="stage",
                                               name="ps")
                            nc.tensor.matmul(ps[:], cs["iw_one"][:], usb[p][:],
                                             start=True, stop=False,
                                             skip_group_check=True)
                            nc.tensor.matmul(ps[:], gw_stage[s][:], hprev[p][:],
                                             start=False, stop=True,
                                             skip_group_check=True)
                            nc.scalar.activation(ht[:], ps[:], TANH, bias=bias)
                        # S += w_s h_s (one accumulation group over all steps)
                        if n == 0 and s == 0:
                            sbank[p] = ps_s.tile([128, CH], f32, tag=f"S{p}",
                                                 name=f"S{p}")
                        nc.tensor.matmul(sbank[p][:], iw_stage[s][:], ht[:],
                                         start=(n == 0 and s == 0),
                                         stop=(last and s == 3),
                                         skip_group_check=True)
                        if s == 2 and not last:
                            # mid-group snapshot: S_{n-1} + h1 + 2h2 + 2h3
                            s123[p] = spool.tile([128, CH], bf16, tag=f"s123_{p}",
                                                 name=f"s123_{p}")
                            nc.vector.tensor_copy(s123[p][:], sbank[p][:])
                            # seed next P off-chain
                            pnext[p] = ps_p.tile([128, CH], f32, tag=f"P{p}",
                                                 name=f"Pn{p}")
                            nc.tensor.matmul(pnext[p][:], cs["iw_one"][:],
                                             u0sb[p][:], start=True, stop=False,
                                             skip_group_check=True)
                            nc.tensor.matmul(pnext[p][:], cs["gw_sixth"][:],
                                             s123[p][:], start=False, stop=False,
                                             skip_group_check=True)
                        if s == 3:
                            h4t[p] = ht
                        hprev[p] = ht
                # step boundary: only (h/6) G^T h4 sits on the tanh chain
                for p in range(NPAIR):
                    if not last:
                        nc.tensor.matmul(pnext[p][:], cs["gw_sixth"][:],
                                         h4t[p][:], start=False, stop=True,
                                         skip_group_check=True)
                        pbank[p] = pnext[p]

            # ---- finale: z = x + W2aug^T [S; 1] ----
            # Split between two engine paths so neither becomes a serial
            # tail: half use a DVE tensor-add (merges the mandatory
            # PSUM->SBUF copy with the skip connection); half add x via a
            # PE identity matmul and copy out on ACT (idle post-RK4).
            # zs tiles hold a full [128, 2048] d-slice so z leaves in eight
            # 1MB DMAs on the SP HWDGE ring.
            for c in range(NCHUNK):
                p, half = c // 2, c % 2
                nc.scalar.copy(sbs[c][0:HID, :],
                               sbank[p][half * HID:(half + 1) * HID, :])
            for m in range(KD):
                zs = zspool.tile([128, TOK], f32, tag="zs", name="zs")
                for c in range(NCHUNK):
                    pz = ps_z.tile([128, CH], f32, tag="pz", name="pz")
                    dve_path = True
                    nc.tensor.matmul(pz[:],
                                     cs["w2aug"][:, m * 128:(m + 1) * 128],
                                     sbs[c][:], start=True, stop=dve_path,
                                     skip_group_check=True)
                    zslice = zs[:, c * CH:(c + 1) * CH]
                    if dve_path:
                        nc.vector.tensor_add(zslice, pz[:], xsl(m, c))
                    else:
                        nc.tensor.matmul(pz[:], cs["iw_one"][:], xsl(m, c),
                                         start=False, stop=True,
                                         skip_group_check=True)
                        nc.scalar.copy(zslice, pz[:])
                zeng = nc.sync if m % 2 == 0 else nc.scalar
                zeng.dma_start(out=zt[m * 128:(m + 1) * 128, :], in_=zs[:])

        body.consts_loaded = False
        if loop_iters is None:
            body()
            for _ in range(unroll - 1):
                body.consts_loaded = True
                body()
        else:
            # timing variant: load the deferred consts once, outside the loop
            for i, name in enumerate(lazy):
                eng = dma_engines[i % len(dma_engines)]
                eng.dma_start(out=cs[name][:], in_=dr[name][:])
            for c in range(NCHUNK):
                nc.sync.dma_start(out=sbs[c][HID:HID + 1, :],
                                  in_=dr["onesrow"][:])
            body.consts_loaded = True
            with tc.For_i(0, loop_iters, 1, staggered_reset=staggered) as iv:
                for _ in range(unroll):
                    body(iv)

    nc.compile()
    return nc


_CACHE = {}


def _get_nc():
    if "nc" not in _CACHE:
        _CACHE["nc"] = build_program()
    return _CACHE["nc"]


def prep_xt(xb):
    """[2048, 1024] float x -> [128, KD*TOK] bf16 with k-chunk k at columns
    k*TOK:(k+1)*TOK (single-DMA layout)."""
    xtb = np.asarray(xb, np.float32).T  # [1024, 2048]
    xtb = xtb.reshape(KD, 128, TOK).transpose(1, 0, 2).reshape(128, KD * TOK)
    return _bf16(xtb)


def kernel(x, W1, b1, W2, b2):
    from concourse.bass_utils import run_bass_kernel_spmd

    x = np.asarray(x, np.float32)
    consts = host_prep(W1, b1, W2, b2)
    nc = _get_nc()

    in_maps = []
    for b in range(N_CORES):
        m = dict(consts)
        m["xt"] = prep_xt(x[b])
        in_maps.append(m)

    res = run_bass_kernel_spmd(nc, in_maps, list(range(N_CORES)))
    out = np.stack([res.results[b]["zt"].T for b in range(N_CORES)], axis=0)
    return np.ascontiguousarray(out, np.float32)
